# revision 44
# baseline (speedup 1.0000x reference)
"""DiversityAttention on 8 TRN2 NeuronCores (Bass/Tile), bf16/fp8 edition.

Sharding: data-parallel over batch (B=2) x tensor-parallel over heads
(16 heads -> 4 groups of 4). core = (b, g), b = core // 4, g = core % 4.
Each core computes full attention for its 4 heads over its batch and a
partial out-projection [S, HIDDEN]; the host sums the 4 partials per
batch and adds bo.

Device-side formulation (keys-on-partitions / "S^T" orientation):
  qT = (Wq/sqrt(dh) @ x^T + bq')  [64h, S]  bf16
  kT = (Wk @ x^T + bk)            [64h, S]  bf16
  vT = (Wv @ x^T + bv) then PE-transposed to V [S, 64h] bf16 (+ ones col)
  xh8 = fp8(64 * x^T / max(||x||, eps))  (host-precomputed)
  per (qb, kt):
     sim_psum[k,q] = xh8^T xh8   (fp8 DoubleRow matmuls, 2 chunks/pass)
     En = exp(-gamma/4096 * sim_psum)            (ACT, bf16)
     per head: sc_psum[k,q] = kT^T qT            (bf16 matmul)
               Es = exp(sc_psum)                 (ACT, bf16)
               P  = Es * En                      (DVE 2x bf16)
  ctx^T[d,q] (+sums row) = sum_k [V|1]^T P       (bf16 matmul, PSUM accum)
  ctx normalized via reciprocal_approx_fast on an SBUF copy of sums
  out[q,o] partial = ctxT^T @ WoT (bf16)  -> DMA to DRAM (f32)
"""

import math
import os
import sys

import numpy as np

for _p in ("/opt/trn_rl_repo",):
    if _p not in sys.path and os.path.isdir(_p):
        sys.path.insert(0, _p)

os.environ.setdefault("MYCRO_LOCAL_CACHE", "1")

import ml_dtypes

import concourse.bass as bass
import concourse.tile as tile
from concourse import bacc, mybir
from concourse.bass_utils import run_bass_kernel_spmd
from concourse.masks import make_identity


def _install_ntff_hook():
    """Provide antenv.axon_hooks (NTFF profiling registry) if the image
    lacks it, mirroring trn_agent_boot's ctypes hook. No-op on failure."""
    try:
        import antenv.axon_hooks  # noqa: F401
        return
    except ImportError:
        pass
    try:
        import contextlib
        import ctypes
        import types

        so_path = "/opt/axon/libaxon_pjrt.so"
        if not os.path.exists(so_path):
            return
        lib = ctypes.CDLL(so_path)
        if not hasattr(lib, "axon_start_nrt_profile"):
            return
        lib.axon_start_nrt_profile.argtypes = [
            ctypes.POINTER(ctypes.c_int64), ctypes.c_size_t]
        lib.axon_start_nrt_profile.restype = ctypes.c_int64
        lib.axon_stop_nrt_profile.argtypes = [ctypes.c_char_p]
        lib.axon_stop_nrt_profile.restype = ctypes.c_int64

        @contextlib.contextmanager
        def _hook(output_dir, device_ids):
            import jax
            jax.devices()
            if device_ids:
                ids = (ctypes.c_int64 * len(device_ids))(*device_ids)
                rc = lib.axon_start_nrt_profile(ids, len(device_ids))
            else:
                rc = lib.axon_start_nrt_profile(None, 0)
            if rc != 0:
                raise RuntimeError(f"axon_start_nrt_profile rc={rc}")
            try:
                yield
            finally:
                n = lib.axon_stop_nrt_profile(str(output_dir).encode())
                print(f"ntff profile: {n} file(s) -> {output_dir}",
                      file=sys.stderr)

        mod = types.ModuleType("antenv.axon_hooks")
        _state = {"hook": _hook}
        mod.set_axon_ntff_profile_hook = lambda h: _state.__setitem__("hook", h)
        mod.get_axon_ntff_profile_hook = lambda: _state["hook"]
        sys.modules["antenv.axon_hooks"] = mod
        import antenv
        antenv.axon_hooks = mod
    except Exception:
        pass


_install_ntff_hook()

F32 = mybir.dt.float32
BF16 = mybir.dt.bfloat16
FP8 = mybir.dt.float8e4
ALU = mybir.AluOpType
ACT_EXP = mybir.ActivationFunctionType.Exp
ACT_COPY = mybir.ActivationFunctionType.Copy
DR = mybir.MatmulPerfMode.DoubleRow

# Problem constants (hardcoded per contract).
HIDDEN = 1024
HEADS = 16
HEAD_DIM = 64
GAMMA = 0.5
B, S = 2, 2048
N_CORES = 8
GROUPS = N_CORES // B  # head groups per batch
HPC = HEADS // GROUPS  # heads per core
LAG = 4  # kt software-pipeline lag between P and ctx matmul
FP8_SCALE = 64.0  # host scales normalized x by this before fp8 cast
VW = 66  # v2 padded row width (64 dims + ones col + pad)
SIM_DR = True  # fp8 DoubleRow for the sim matmuls


def emit_kernel(tc, aps, *, S_, C_, HPC_, QB, with_mask, debug_taps=False):
    """Emit the per-core kernel. aps: dict of dram APs."""
    nc = tc.nc
    CT = C_ // 128          # contraction chunks over hidden
    PAIRS = HPC_ // 2       # head pairs (128-channel chunks)
    NKT = S_ // 128         # key tiles
    NQB = S_ // QB          # query blocks
    PB = min(512, S_)       # projection free-block width
    NPB = S_ // PB
    OB_W = min(512, C_)     # out-projection free-block width
    NOB = C_ // OB_W
    D2 = HPC_ * HEAD_DIM
    neg_gamma_scale = -GAMMA / (FP8_SCALE * FP8_SCALE)

    xbf_d = aps["xbf"]; xh8_d = aps["xh8"]
    wq_d = aps["wq"]; wk_d = aps["wk"]; wv_d = aps["wv"]; wo_d = aps["wo"]
    bq_d = aps["bq"]; bk_d = aps["bk"]; bv_d = aps["bv"]
    out_d = aps["out"]
    m01_d = aps.get("mask01")

    from contextlib import ExitStack
    stack = ExitStack()
    consts = stack.enter_context(tc.tile_pool(name="consts", bufs=1))
    xpool = stack.enter_context(tc.tile_pool(name="xpool", bufs=1))
    projpool = stack.enter_context(tc.tile_pool(name="projpool", bufs=1))

    identity = consts.tile([128, 128], BF16)
    make_identity(nc, identity)

    wo_sb = consts.tile([128, PAIRS, C_], BF16)

    # x^T in fp8 (sim), host-precast; the bf16 copy for projections is
    # phase-1-scoped (freed before the main loop needs SBUF for en_all)
    xh8_sb = xpool.tile([128, CT, S_], FP8)

    # projections
    qT_sb = projpool.tile([128, PAIRS, S_], BF16)
    kT_sb = projpool.tile([128, PAIRS, S_], BF16)
    v2_sb = projpool.tile([128, HPC_, NKT, VW], BF16)
    # all En blocks [k-tile, q-block] kept resident: below-diagonal blocks
    # are PE-transposed mirrors of computed ones (En is symmetric)
    en_all = projpool.tile([128, NKT, NQB, QB], BF16)

    with tc.tile_pool(name="wpool", bufs=1) as wpool, \
         tc.tile_pool(name="vstage", bufs=1) as vstage, \
         tc.tile_pool(name="ph1psum", bufs=2, space="PSUM") as prj_ps, \
         tc.tile_pool(name="tppsum", bufs=4, space="PSUM") as tp_ps:
        xbf_sb = wpool.tile([128, CT, S_], BF16)
        wq_sb = wpool.tile([128, CT, D2], BF16)
        wk_sb = wpool.tile([128, CT, D2], BF16)
        wv_sb = wpool.tile([128, CT, D2], BF16)
        bq_sb = wpool.tile([128, PAIRS, 1], F32)
        bk_sb = wpool.tile([128, PAIRS, 1], F32)
        bv_sb = wpool.tile([128, PAIRS, 1], F32)
        # DMA order = arrival order: wq first so projections start early,
        # xh8/wo last (needed only by the main loop / out-projection).
        nc.sync.dma_start(out=wq_sb, in_=wq_d.rearrange("(t p) m -> p t m", p=128))
        nc.sync.dma_start(out=bq_sb, in_=bq_d.rearrange("(j p) one -> p j one", p=128))
        for c in range(CT):
            nc.sync.dma_start(out=xbf_sb[:, c, :],
                              in_=xbf_d[c * 128:(c + 1) * 128, :])
        for w_sb, b_sb, w_d, b_d in ((wk_sb, bk_sb, wk_d, bk_d),
                                     (wv_sb, bv_sb, wv_d, bv_d)):
            nc.sync.dma_start(out=w_sb, in_=w_d.rearrange("(t p) m -> p t m", p=128))
            nc.sync.dma_start(out=b_sb,
                              in_=b_d.rearrange("(j p) one -> p j one", p=128))
        for c in range(CT):
            nc.sync.dma_start(out=xh8_sb[:, c, :],
                              in_=xh8_d[c * 128:(c + 1) * 128, :])
        nc.sync.dma_start(out=wo_sb, in_=wo_d.rearrange("(j p) o -> p j o", p=128))

        vT_sb = vstage.tile([128, PAIRS, S_], BF16)
        for w_sb, b_sb, dest in (
            (wq_sb, bq_sb, qT_sb),
            (wk_sb, bk_sb, kT_sb),
            (wv_sb, bv_sb, vT_sb),
        ):
            for nb in range(NPB):
                pss = [prj_ps.tile([128, PB], F32, tag=f"prj{j}",
                                   name=f"prj_{dest.tensor.name}_{nb}_{j}")
                       for j in range(PAIRS)]
                for c in range(CT):
                    for j in range(PAIRS):
                        nc.tensor.matmul(
                            pss[j],
                            w_sb[:, c, j * 128:(j + 1) * 128],
                            xbf_sb[:, c, nb * PB:(nb + 1) * PB],
                            start=(c == 0),
                            stop=(c == CT - 1),
                        )
                for j in range(PAIRS):
                    nc.vector.tensor_scalar_add(
                        dest[:, j, nb * PB:(nb + 1) * PB], pss[j], b_sb[:, j, :]
                    )

        if debug_taps:
            nc.sync.dma_start(out=aps["dbg_qT"], in_=qT_sb)
            nc.sync.dma_start(out=aps["dbg_kT"], in_=kT_sb)

        # V: PE-transpose vT (bf16) -> [keys, d] layout, 2 heads per tile
        for j in range(PAIRS):
            for t in range(NKT):
                tp = tp_ps.tile([128, 128], BF16, tag="tp")
                nc.tensor.transpose(tp, vT_sb[:, j, t * 128:(t + 1) * 128], identity)
                nc.vector.tensor_copy(
                    v2_sb[:, 2 * j:2 * j + 2, t, 0:HEAD_DIM],
                    tp.rearrange("p (h d) -> p h d", h=2),
                )
        nc.vector.memset(v2_sb[:, :, :, HEAD_DIM:HEAD_DIM + 1], 1.0)
        if debug_taps:
            nc.sync.dma_start(out=aps["dbg_v2"],
                              in_=v2_sb[:, :, :, 0:HEAD_DIM + 1])

    # --- main loop (phase 2) ---
    ctxT2_sb = projpool.tile([128, PAIRS, S_], BF16)
    ptpool = stack.enter_context(tc.tile_pool(name="ptpool", bufs=22))
    espool = stack.enter_context(tc.tile_pool(name="espool", bufs=4))
    enpool = stack.enter_context(tc.tile_pool(name="enpool", bufs=3))
    smallpool = stack.enter_context(tc.tile_pool(name="smallpool", bufs=2))
    mpool = (stack.enter_context(tc.tile_pool(name="mpool", bufs=2))
             if m01_d is not None else None)

    with tc.tile_pool(name="simpsum", bufs=1, space="PSUM") as simp, \
         tc.tile_pool(name="scpsum", bufs=3, space="PSUM") as scp, \
         tc.tile_pool(name="ctxpsum", bufs=1, space="PSUM") as ctxp:

        def emit_ctx(ctx_ps, kt, pts, first, last):
            for h in range(HPC_):
                nc.tensor.matmul(
                    ctx_ps[h],
                    v2_sb[:, h, kt, 0:HEAD_DIM + 1],
                    pts[h],
                    start=first,
                    stop=last,
                    skip_group_check=True,
                )

        def emit_division_head(qb, ctx_ps, h):
            j, hi = divmod(h, 2)
            s_sb = smallpool.tile([1, QB], F32, tag=f"ssb{h % 2}",
                                  name=f"ssb_{qb}_{h}")
            nc.vector.tensor_copy(s_sb, ctx_ps[h][HEAD_DIM:HEAD_DIM + 1, :])
            r0 = smallpool.tile([1, QB], F32, tag=f"r0{h % 2}",
                                name=f"r0_{qb}_{h}")
            nc.vector.reciprocal_approx_fast(r0, s_sb)
            if debug_taps and qb == 0 and h == 0:
                nc.sync.dma_start(out=aps["dbg_r0"], in_=r0)
            rb = smallpool.tile([HEAD_DIM, QB], F32, tag="rb")
            nc.gpsimd.partition_broadcast(rb, r0, channels=HEAD_DIM)
            nc.vector.tensor_mul(
                ctxT2_sb[hi * 64:hi * 64 + 64, j, qb * QB:(qb + 1) * QB],
                ctx_ps[h][0:HEAD_DIM, :],
                rb,
            )

        def emit_division(qb, ctx_ps):
            for h in range(HPC_):
                emit_division_head(qb, ctx_ps, h)

        prev_div = None
        for qb in range(NQB):
            ctx_ps = [ctxp.tile([HEAD_DIM + 1, QB], F32, tag=f"ctx{h}",
                                name=f"ctx_{qb}_{h}")
                      for h in range(HPC_)]
            # interleave mirrored (PE-transpose) blocks between computed
            # (DoubleRow matmul) blocks so HAM always sees matmul traffic
            # (transpose-mode doesn't count as PE-busy for the clock gate)
            kts_computed = list(range(4 * qb, NKT))
            kts_mirror = list(range(0, 4 * qb))
            kt_order = []
            mi = 0
            acc = 0.0
            ratio = len(kts_mirror) / len(kts_computed)
            for ckt in kts_computed:
                kt_order.append(ckt)
                acc += ratio
                while acc >= 1.0:
                    kt_order.append(kts_mirror[mi])
                    mi += 1
                    acc -= 1.0
            kt_order.extend(kts_mirror[mi:])
            npop = 0
            pending = []
            for idx, kt in enumerate(kt_order):
                if prev_div is not None and idx < HPC_:
                    emit_division_head(prev_div[0], prev_div[1], idx)
                    if idx == HPC_ - 1:
                        prev_div = None
                sp = simp.tile([128, QB], F32, tag="sim")
                if kt < 4 * qb:
                    # below-diagonal: mirror of computed En via PE transpose
                    # (En symmetric). M[:, 128i:128(i+1)] = src_i^T with
                    # src_i = en_all[:, 4qb+i, kt//4, (kt%4)*128 : +128]
                    spb = sp.bitcast(BF16)
                    for i in range(QB // 128):
                        nc.tensor.matmul(
                            spb[:, i * 128:(i + 1) * 128],
                            en_all[:, 4 * qb + i, kt // 4,
                                   (kt % 4) * 128:(kt % 4 + 1) * 128],
                            identity,
                            is_transpose=True,
                            skip_group_check=True,
                        )
                    en = enpool.tile([128, QB], BF16, tag="en")
                    nc.scalar.activation(out=en, in_=spb[:, 0:QB],
                                         func=ACT_COPY)
                else:
                    # sim via fp8 DoubleRow: 2 hidden-chunks per pass
                    for c2 in range(CT // 2):
                        nc.tensor.matmul(
                            sp,
                            xh8_sb[:, 2 * c2:2 * c2 + 2, kt * 128:(kt + 1) * 128],
                            xh8_sb[:, 2 * c2:2 * c2 + 2, qb * QB:(qb + 1) * QB],
                            start=(c2 == 0),
                            stop=(c2 == CT // 2 - 1),
                            perf_mode=DR,
                        )
                    en = en_all[:, kt, qb, :]
                    nc.scalar.activation(out=en, in_=sp, func=ACT_EXP,
                                         scale=neg_gamma_scale)
                if debug_taps and qb == 0 and kt == 0:
                    nc.sync.dma_start(out=aps["dbg_en0"], in_=en)
                if m01_d is not None:
                    m_sb = mpool.tile([128, QB], BF16, tag="msk")
                    nc.sync.dma_start(
                        out=m_sb,
                        in_=m01_d[kt * 128:(kt + 1) * 128, qb * QB:(qb + 1) * QB],
                    )
                    nc.vector.tensor_mul(en, en, m_sb)
                pts = []
                for h in range(HPC_):
                    j, hi = divmod(h, 2)
                    pr = slice(hi * 64, hi * 64 + 64)
                    sc_t = scp.tile([128, QB], F32, tag="sc")
                    nc.tensor.matmul(
                        sc_t,
                        kT_sb[pr, j, kt * 128:(kt + 1) * 128],
                        qT_sb[pr, j, qb * QB:(qb + 1) * QB],
                        start=True,
                        stop=True,
                    )
                    es = espool.tile([128, QB], BF16, tag="es")
                    nc.scalar.activation(out=es, in_=sc_t, func=ACT_EXP)
                    pt = ptpool.tile([128, QB], BF16, tag="pt")
                    nc.vector.tensor_mul(pt, es, en)
                    if debug_taps and qb == 0 and kt == 0 and h == 0:
                        nc.sync.dma_start(out=aps["dbg_pt0"], in_=pt)
                    pts.append(pt)
                pending.append((kt, pts))
                if len(pending) > LAG:
                    k0, p0 = pending.pop(0)
                    emit_ctx(ctx_ps, k0, p0, npop == 0, npop == NKT - 1)
                    npop += 1
            if qb < NQB - 1:
                for k0, p0 in pending:
                    emit_ctx(ctx_ps, k0, p0, npop == 0, npop == NKT - 1)
                    npop += 1
                prev_div = (qb, ctx_ps)
            else:
                # final qb: drain head-by-head so divisions start ASAP and
                # the out-projection isn't gated on one long division tail
                for h in range(HPC_):
                    for j, (k0, p0) in enumerate(pending):
                        nc.tensor.matmul(
                            ctx_ps[h],
                            v2_sb[:, h, k0, 0:HEAD_DIM + 1],
                            p0[h],
                            start=(npop + j == 0),
                            stop=(npop + j == NKT - 1),
                            skip_group_check=True,
                        )
                    emit_division_head(qb, ctx_ps, h)
                prev_div = None

    if debug_taps:
        nc.sync.dma_start(out=aps["dbg_ctxT2"], in_=ctxT2_sb)

    # --- out-projection (phase 3) ---
    with tc.tile_pool(name="outpsum", bufs=4, space="PSUM") as outp, \
         tc.tile_pool(name="outstg", bufs=4) as outstg:
        for qt in range(S_ // 128):
            for ob in range(NOB):
                op = outp.tile([128, OB_W], F32, tag="op")
                for j in range(PAIRS):
                    nc.tensor.matmul(
                        op,
                        ctxT2_sb[:, j, qt * 128:(qt + 1) * 128],
                        wo_sb[:, j, ob * OB_W:(ob + 1) * OB_W],
                        start=(j == 0),
                        stop=(j == PAIRS - 1),
                    )
                ostg = outstg.tile([128, OB_W], F32, tag="ostg")
                nc.vector.tensor_copy(ostg, op)
                nc.sync.dma_start(
                    out=out_d[qt * 128:(qt + 1) * 128, ob * OB_W:(ob + 1) * OB_W],
                    in_=ostg,
                )

    stack.close()


def build_nc(*, S_=S, C_=HIDDEN, HPC_=HPC, QB=512, with_mask=False,
             enable_asserts=False, debug_taps=False):
    nc = bacc.Bacc(
        "TRN2", target_bir_lowering=False, debug=False,
        enable_asserts=enable_asserts,
    )
    D2 = HPC_ * HEAD_DIM
    PAIRS = HPC_ // 2
    NKT = S_ // 128
    aps = {}
    aps["xbf"] = nc.dram_tensor("xbf", [C_, S_], BF16, kind="ExternalInput").ap()
    aps["xh8"] = nc.dram_tensor("xh8", [C_, S_], FP8, kind="ExternalInput").ap()
    for n in ("wq", "wk", "wv"):
        aps[n] = nc.dram_tensor(n, [C_, D2], BF16, kind="ExternalInput").ap()
    aps["wo"] = nc.dram_tensor("wo", [D2, C_], BF16, kind="ExternalInput").ap()
    for n in ("bq", "bk", "bv"):
        aps[n] = nc.dram_tensor(n, [D2, 1], F32, kind="ExternalInput").ap()
    if with_mask:
        aps["mask01"] = nc.dram_tensor(
            "mask01", [S_, S_], BF16, kind="ExternalInput").ap()
    aps["out"] = nc.dram_tensor("out", [S_, C_], F32, kind="ExternalOutput").ap()
    if debug_taps:
        aps["dbg_qT"] = nc.dram_tensor(
            "dbg_qT", [128, PAIRS, S_], BF16, kind="ExternalOutput").ap()
        aps["dbg_kT"] = nc.dram_tensor(
            "dbg_kT", [128, PAIRS, S_], BF16, kind="ExternalOutput").ap()
        aps["dbg_v2"] = nc.dram_tensor(
            "dbg_v2", [128, HPC_, NKT, HEAD_DIM + 1], BF16,
            kind="ExternalOutput").ap()
        aps["dbg_ctxT2"] = nc.dram_tensor(
            "dbg_ctxT2", [128, PAIRS, S_], BF16, kind="ExternalOutput").ap()
        aps["dbg_en0"] = nc.dram_tensor(
            "dbg_en0", [128, QB], BF16, kind="ExternalOutput").ap()
        aps["dbg_pt0"] = nc.dram_tensor(
            "dbg_pt0", [128, QB], BF16, kind="ExternalOutput").ap()
        aps["dbg_r0"] = nc.dram_tensor(
            "dbg_r0", [1, QB], F32, kind="ExternalOutput").ap()

    with tile.TileContext(nc) as tc:
        emit_kernel(tc, aps, S_=S_, C_=C_, HPC_=HPC_, QB=QB,
                    with_mask=with_mask, debug_taps=debug_taps)
    nc.compile()
    return nc


def host_prepare(x, attn_mask, Wq, bq, Wk, bk, Wv, bv, Wo, bo, *,
                 S_=S, C_=HIDDEN, HPC_=HPC, n_cores=N_CORES):
    """Build the per-core input maps. Returns (in_maps, with_mask)."""
    bf = ml_dtypes.bfloat16
    f8 = ml_dtypes.float8_e4m3fn
    x = np.asarray(x, np.float32)
    B_ = x.shape[0]
    groups = n_cores // B_
    Wq = np.asarray(Wq, np.float32); Wk = np.asarray(Wk, np.float32)
    Wv = np.asarray(Wv, np.float32); Wo = np.asarray(Wo, np.float32)
    bq = np.asarray(bq, np.float32); bk = np.asarray(bk, np.float32)
    bv = np.asarray(bv, np.float32)

    inv_sqrt_d = 1.0 / math.sqrt(HEAD_DIM)
    WqT = np.ascontiguousarray((Wq * inv_sqrt_d).T).astype(bf)  # [C, C]
    WkT = np.ascontiguousarray(Wk.T).astype(bf)
    WvT = np.ascontiguousarray(Wv.T).astype(bf)
    WoT = np.ascontiguousarray(Wo.T).astype(bf)                 # [C(c), C(o)]
    bq = bq * inv_sqrt_d

    mask = np.asarray(attn_mask)
    with_mask = bool(mask.any())
    mask01 = None
    if with_mask:
        # reference: where(mask, -inf) -> multiplicative 0/1 on P
        mask01 = np.where(mask, 0.0, 1.0).astype(bf)
        mask01 = np.ascontiguousarray(mask01.T)  # [k, q]

    in_maps = []
    for core in range(n_cores):
        b, g = divmod(core, groups)
        xb = x[b]                                   # [S, C]
        xT = np.ascontiguousarray(xb.T)             # [C, S]
        norms = np.linalg.norm(xb, axis=1)          # [S]
        scale = (FP8_SCALE / np.maximum(norms, 1e-12)).astype(np.float32)
        xh8 = (xT * scale[None, :]).astype(f8)
        ch = slice(g * HPC_ * HEAD_DIM, (g + 1) * HPC_ * HEAD_DIM)
        m = {
            "xbf": xT.astype(bf),
            "xh8": xh8,
            "wq": np.ascontiguousarray(WqT[:, ch]),
            "wk": np.ascontiguousarray(WkT[:, ch]),
            "wv": np.ascontiguousarray(WvT[:, ch]),
            "wo": np.ascontiguousarray(WoT[ch, :]),
            "bq": np.ascontiguousarray(bq[ch]).reshape(-1, 1),
            "bk": np.ascontiguousarray(bk[ch]).reshape(-1, 1),
            "bv": np.ascontiguousarray(bv[ch]).reshape(-1, 1),
        }
        if with_mask:
            m["mask01"] = mask01
        in_maps.append(m)
    return in_maps, with_mask


_NC_CACHE = {}


def _get_nc(with_mask):
    key = with_mask
    if key not in _NC_CACHE:
        _NC_CACHE[key] = build_nc(with_mask=with_mask)
    return _NC_CACHE[key]


LAST_RESULTS = None


def kernel(**inputs):
    global LAST_RESULTS
    in_maps, with_mask = host_prepare(
        inputs["x"], inputs["attn_mask"],
        inputs["Wq"], inputs["bq"], inputs["Wk"], inputs["bk"],
        inputs["Wv"], inputs["bv"], inputs["Wo"], inputs["bo"],
    )
    nc = _get_nc(with_mask)
    res = run_bass_kernel_spmd(nc, in_maps, core_ids=list(range(N_CORES)))
    LAST_RESULTS = res
    bo = np.asarray(inputs["bo"], np.float32)
    out = np.zeros((B, S, HIDDEN), np.float32)
    groups = N_CORES // B
    for core in range(N_CORES):
        b = core // groups
        out[b] += res.results[core]["out"]
    out += bo[None, None, :]
    return out


# revision 46
# speedup vs baseline: 1.1647x; 1.1647x over previous
"""DiversityAttention on 8 TRN2 NeuronCores (Bass/Tile), bf16/fp8 edition.

Sharding: data-parallel over batch (B=2) x tensor-parallel over heads
(16 heads -> 4 groups of 4). core = (b, g), b = core // 4, g = core % 4.
Each core computes full attention for its 4 heads over its batch and a
partial out-projection [S, HIDDEN]; the host sums the 4 partials per
batch and adds bo.

Device-side formulation (keys-on-partitions / "S^T" orientation):
  qT = (Wq/sqrt(dh) @ x^T + bq')  [64h, S]  bf16
  kT = (Wk @ x^T + bk)            [64h, S]  bf16
  vT = (Wv @ x^T + bv) then PE-transposed to V [S, 64h] bf16 (+ ones col)
  xh8 = fp8(64 * x^T / max(||x||, eps))  (host-precomputed)
  per (qb, kt):
     sim_psum[k,q] = xh8^T xh8   (fp8 DoubleRow matmuls, 2 chunks/pass)
     En = exp(-gamma/4096 * sim_psum)            (ACT, bf16)
     per head: sc_psum[k,q] = kT^T qT            (bf16 matmul)
               Es = exp(sc_psum)                 (ACT, bf16)
               P  = Es * En                      (DVE 2x bf16)
  ctx^T[d,q] (+sums row) = sum_k [V|1]^T P       (bf16 matmul, PSUM accum)
  ctx normalized via reciprocal_approx_fast on an SBUF copy of sums
  out[q,o] partial = ctxT^T @ WoT (bf16)  -> DMA to DRAM (f32)
"""

import math
import os
import sys

import numpy as np

for _p in ("/opt/trn_rl_repo",):
    if _p not in sys.path and os.path.isdir(_p):
        sys.path.insert(0, _p)

os.environ.setdefault("MYCRO_LOCAL_CACHE", "1")

import ml_dtypes

import concourse.bass as bass
import concourse.tile as tile
from concourse import bacc, mybir
from concourse.bass_utils import run_bass_kernel_spmd
from concourse.masks import make_identity


def _install_ntff_hook():
    """Provide antenv.axon_hooks (NTFF profiling registry) if the image
    lacks it, mirroring trn_agent_boot's ctypes hook. No-op on failure."""
    try:
        import antenv.axon_hooks  # noqa: F401
        return
    except ImportError:
        pass
    try:
        import contextlib
        import ctypes
        import types

        so_path = "/opt/axon/libaxon_pjrt.so"
        if not os.path.exists(so_path):
            return
        lib = ctypes.CDLL(so_path)
        if not hasattr(lib, "axon_start_nrt_profile"):
            return
        lib.axon_start_nrt_profile.argtypes = [
            ctypes.POINTER(ctypes.c_int64), ctypes.c_size_t]
        lib.axon_start_nrt_profile.restype = ctypes.c_int64
        lib.axon_stop_nrt_profile.argtypes = [ctypes.c_char_p]
        lib.axon_stop_nrt_profile.restype = ctypes.c_int64

        @contextlib.contextmanager
        def _hook(output_dir, device_ids):
            import jax
            jax.devices()
            if device_ids:
                ids = (ctypes.c_int64 * len(device_ids))(*device_ids)
                rc = lib.axon_start_nrt_profile(ids, len(device_ids))
            else:
                rc = lib.axon_start_nrt_profile(None, 0)
            if rc != 0:
                raise RuntimeError(f"axon_start_nrt_profile rc={rc}")
            try:
                yield
            finally:
                n = lib.axon_stop_nrt_profile(str(output_dir).encode())
                print(f"ntff profile: {n} file(s) -> {output_dir}",
                      file=sys.stderr)

        mod = types.ModuleType("antenv.axon_hooks")
        _state = {"hook": _hook}
        mod.set_axon_ntff_profile_hook = lambda h: _state.__setitem__("hook", h)
        mod.get_axon_ntff_profile_hook = lambda: _state["hook"]
        sys.modules["antenv.axon_hooks"] = mod
        import antenv
        antenv.axon_hooks = mod
    except Exception:
        pass


_install_ntff_hook()

F32 = mybir.dt.float32
BF16 = mybir.dt.bfloat16
FP8 = mybir.dt.float8e4
ALU = mybir.AluOpType
ACT_EXP = mybir.ActivationFunctionType.Exp
ACT_COPY = mybir.ActivationFunctionType.Copy
DR = mybir.MatmulPerfMode.DoubleRow

# Problem constants (hardcoded per contract).
HIDDEN = 1024
HEADS = 16
HEAD_DIM = 64
GAMMA = 0.5
B, S = 2, 2048
N_CORES = 8
GROUPS = N_CORES // B  # head groups per batch
HPC = HEADS // GROUPS  # heads per core
LAG = 4  # kt software-pipeline lag between P and ctx matmul
FP8_SCALE = 64.0  # host scales normalized x by this before fp8 cast
VW = 66  # v2 padded row width (64 dims + ones col + pad)
SIM_DR = True  # fp8 DoubleRow for the sim matmuls


def emit_kernel(tc, aps, *, S_, C_, HPC_, QB, with_mask, debug_taps=False):
    """Emit the per-core kernel. aps: dict of dram APs."""
    nc = tc.nc
    CT = C_ // 128          # contraction chunks over hidden
    PAIRS = HPC_ // 2       # head pairs (128-channel chunks)
    NKT = S_ // 128         # key tiles
    NQB = S_ // QB          # query blocks
    PB = min(512, S_)       # projection free-block width
    NPB = S_ // PB
    OB_W = min(512, C_)     # out-projection free-block width
    NOB = C_ // OB_W
    D2 = HPC_ * HEAD_DIM
    neg_gamma_scale = -GAMMA / (FP8_SCALE * FP8_SCALE)

    xbf_d = aps["xbf"]; xh8_d = aps["xh8"]
    wq_d = aps["wq"]; wk_d = aps["wk"]; wv_d = aps["wv"]; wo_d = aps["wo"]
    bq_d = aps["bq"]; bk_d = aps["bk"]; bv_d = aps["bv"]
    out_d = aps["out"]
    m01_d = aps.get("mask01")

    from contextlib import ExitStack
    stack = ExitStack()
    consts = stack.enter_context(tc.tile_pool(name="consts", bufs=1))
    xpool = stack.enter_context(tc.tile_pool(name="xpool", bufs=1))
    projpool = stack.enter_context(tc.tile_pool(name="projpool", bufs=1))

    identity = consts.tile([128, 128], BF16)
    make_identity(nc, identity)

    wo_sb = consts.tile([128, PAIRS, C_], BF16)

    # x^T in bf16 (projections) and fp8 (sim), both host-precast
    xbf_sb = xpool.tile([128, CT, S_], BF16)
    xh8_sb = xpool.tile([128, CT, S_], FP8)

    # projections
    qT_sb = projpool.tile([128, PAIRS, S_], BF16)
    kT_sb = projpool.tile([128, PAIRS, S_], BF16)
    v2_sb = projpool.tile([128, HPC_, NKT, VW], BF16)

    with tc.tile_pool(name="wpool", bufs=1) as wpool, \
         tc.tile_pool(name="vstage", bufs=1) as vstage, \
         tc.tile_pool(name="ph1psum", bufs=2, space="PSUM") as prj_ps, \
         tc.tile_pool(name="tppsum", bufs=4, space="PSUM") as tp_ps:
        wq_sb = wpool.tile([128, CT, D2], BF16)
        wk_sb = wpool.tile([128, CT, D2], BF16)
        wv_sb = wpool.tile([128, CT, D2], BF16)
        bq_sb = wpool.tile([128, PAIRS, 1], F32)
        bk_sb = wpool.tile([128, PAIRS, 1], F32)
        bv_sb = wpool.tile([128, PAIRS, 1], F32)
        # DMA order = arrival order: wq first so projections start early,
        # xh8/wo last (needed only by the main loop / out-projection).
        nc.sync.dma_start(out=wq_sb, in_=wq_d.rearrange("(t p) m -> p t m", p=128))
        nc.sync.dma_start(out=bq_sb, in_=bq_d.rearrange("(j p) one -> p j one", p=128))
        for c in range(CT):
            nc.sync.dma_start(out=xbf_sb[:, c, :],
                              in_=xbf_d[c * 128:(c + 1) * 128, :])
        for w_sb, b_sb, w_d, b_d in ((wk_sb, bk_sb, wk_d, bk_d),
                                     (wv_sb, bv_sb, wv_d, bv_d)):
            nc.sync.dma_start(out=w_sb, in_=w_d.rearrange("(t p) m -> p t m", p=128))
            nc.sync.dma_start(out=b_sb,
                              in_=b_d.rearrange("(j p) one -> p j one", p=128))
        for c in range(CT):
            nc.sync.dma_start(out=xh8_sb[:, c, :],
                              in_=xh8_d[c * 128:(c + 1) * 128, :])
        nc.sync.dma_start(out=wo_sb, in_=wo_d.rearrange("(j p) o -> p j o", p=128))

        vT_sb = vstage.tile([128, PAIRS, S_], BF16)
        for w_sb, b_sb, dest in (
            (wq_sb, bq_sb, qT_sb),
            (wk_sb, bk_sb, kT_sb),
            (wv_sb, bv_sb, vT_sb),
        ):
            for nb in range(NPB):
                pss = [prj_ps.tile([128, PB], F32, tag=f"prj{j}",
                                   name=f"prj_{dest.tensor.name}_{nb}_{j}")
                       for j in range(PAIRS)]
                for c in range(CT):
                    for j in range(PAIRS):
                        nc.tensor.matmul(
                            pss[j],
                            w_sb[:, c, j * 128:(j + 1) * 128],
                            xbf_sb[:, c, nb * PB:(nb + 1) * PB],
                            start=(c == 0),
                            stop=(c == CT - 1),
                        )
                for j in range(PAIRS):
                    nc.vector.tensor_scalar_add(
                        dest[:, j, nb * PB:(nb + 1) * PB], pss[j], b_sb[:, j, :]
                    )

        if debug_taps:
            nc.sync.dma_start(out=aps["dbg_qT"], in_=qT_sb)
            nc.sync.dma_start(out=aps["dbg_kT"], in_=kT_sb)

        # V: PE-transpose vT (bf16) -> [keys, d] layout, 2 heads per tile
        for j in range(PAIRS):
            for t in range(NKT):
                tp = tp_ps.tile([128, 128], BF16, tag="tp")
                nc.tensor.transpose(tp, vT_sb[:, j, t * 128:(t + 1) * 128], identity)
                nc.vector.tensor_copy(
                    v2_sb[:, 2 * j:2 * j + 2, t, 0:HEAD_DIM],
                    tp.rearrange("p (h d) -> p h d", h=2),
                )
        nc.vector.memset(v2_sb[:, :, :, HEAD_DIM:HEAD_DIM + 1], 1.0)
        if debug_taps:
            nc.sync.dma_start(out=aps["dbg_v2"],
                              in_=v2_sb[:, :, :, 0:HEAD_DIM + 1])

    # --- main loop (phase 2) ---
    ctxT2_sb = projpool.tile([128, PAIRS, S_], BF16)
    ptpool = stack.enter_context(tc.tile_pool(name="ptpool", bufs=22))
    espool = stack.enter_context(tc.tile_pool(name="espool", bufs=4))
    enpool = stack.enter_context(tc.tile_pool(name="enpool", bufs=3))
    smallpool = stack.enter_context(tc.tile_pool(name="smallpool", bufs=2))
    mpool = (stack.enter_context(tc.tile_pool(name="mpool", bufs=2))
             if m01_d is not None else None)

    with tc.tile_pool(name="simpsum", bufs=1, space="PSUM") as simp, \
         tc.tile_pool(name="scpsum", bufs=3, space="PSUM") as scp, \
         tc.tile_pool(name="ctxpsum", bufs=1, space="PSUM") as ctxp:

        def emit_ctx(ctx_ps, kt, pts):
            for h in range(HPC_):
                nc.tensor.matmul(
                    ctx_ps[h],
                    v2_sb[:, h, kt, 0:HEAD_DIM + 1],
                    pts[h],
                    start=(kt == 0),
                    stop=(kt == NKT - 1),
                    skip_group_check=True,
                )

        def emit_division_head(qb, ctx_ps, h):
            j, hi = divmod(h, 2)
            s_sb = smallpool.tile([1, QB], F32, tag=f"ssb{h % 2}",
                                  name=f"ssb_{qb}_{h}")
            nc.vector.tensor_copy(s_sb, ctx_ps[h][HEAD_DIM:HEAD_DIM + 1, :])
            r0 = smallpool.tile([1, QB], F32, tag=f"r0{h % 2}",
                                name=f"r0_{qb}_{h}")
            nc.vector.reciprocal_approx_fast(r0, s_sb)
            if debug_taps and qb == 0 and h == 0:
                nc.sync.dma_start(out=aps["dbg_r0"], in_=r0)
            rb = smallpool.tile([HEAD_DIM, QB], F32, tag="rb")
            nc.gpsimd.partition_broadcast(rb, r0, channels=HEAD_DIM)
            nc.vector.tensor_mul(
                ctxT2_sb[hi * 64:hi * 64 + 64, j, qb * QB:(qb + 1) * QB],
                ctx_ps[h][0:HEAD_DIM, :],
                rb,
            )

        def emit_division(qb, ctx_ps):
            for h in range(HPC_):
                emit_division_head(qb, ctx_ps, h)

        prev_div = None
        for qb in range(NQB):
            ctx_ps = [ctxp.tile([HEAD_DIM + 1, QB], F32, tag=f"ctx{h}",
                                name=f"ctx_{qb}_{h}")
                      for h in range(HPC_)]
            pending = []
            for kt in range(NKT):
                if prev_div is not None and kt < HPC_:
                    emit_division_head(prev_div[0], prev_div[1], kt)
                    if kt == HPC_ - 1:
                        prev_div = None
                # sim via fp8 DoubleRow: 2 hidden-chunks per pass
                sp = simp.tile([128, QB], F32, tag="sim")
                if SIM_DR:
                    for c2 in range(CT // 2):
                        nc.tensor.matmul(
                            sp,
                            xh8_sb[:, 2 * c2:2 * c2 + 2, kt * 128:(kt + 1) * 128],
                            xh8_sb[:, 2 * c2:2 * c2 + 2, qb * QB:(qb + 1) * QB],
                            start=(c2 == 0),
                            stop=(c2 == CT // 2 - 1),
                            perf_mode=DR,
                        )
                else:
                    for c in range(CT):
                        nc.tensor.matmul(
                            sp,
                            xh8_sb[:, c, kt * 128:(kt + 1) * 128],
                            xh8_sb[:, c, qb * QB:(qb + 1) * QB],
                            start=(c == 0),
                            stop=(c == CT - 1),
                        )
                pts = []
                en = None
                for h in range(HPC_):
                    j, hi = divmod(h, 2)
                    pr = slice(hi * 64, hi * 64 + 64)
                    sc_t = scp.tile([128, QB], F32, tag="sc")
                    nc.tensor.matmul(
                        sc_t,
                        kT_sb[pr, j, kt * 128:(kt + 1) * 128],
                        qT_sb[pr, j, qb * QB:(qb + 1) * QB],
                        start=True,
                        stop=True,
                    )
                    es = espool.tile([128, QB], BF16, tag="es")
                    nc.scalar.activation(out=es, in_=sc_t, func=ACT_EXP)
                    if h == 0:
                        # emit the En exp AFTER the first head's Es so the
                        # score-bank rotation frees a bank ~0.7us earlier
                        # (es_h0 no longer queues behind en on ACT)
                        en = enpool.tile([128, QB], BF16, tag="en")
                        nc.scalar.activation(out=en, in_=sp, func=ACT_EXP,
                                             scale=neg_gamma_scale)
                        if debug_taps and qb == 0 and kt == 0:
                            nc.sync.dma_start(out=aps["dbg_en0"], in_=en)
                        if m01_d is not None:
                            m_sb = mpool.tile([128, QB], BF16, tag="msk")
                            nc.sync.dma_start(
                                out=m_sb,
                                in_=m01_d[kt * 128:(kt + 1) * 128,
                                          qb * QB:(qb + 1) * QB],
                            )
                            nc.vector.tensor_mul(en, en, m_sb)
                    pt = ptpool.tile([128, QB], BF16, tag="pt")
                    nc.vector.tensor_mul(pt, es, en)
                    if debug_taps and qb == 0 and kt == 0 and h == 0:
                        nc.sync.dma_start(out=aps["dbg_pt0"], in_=pt)
                    pts.append(pt)
                pending.append((kt, pts))
                if len(pending) > LAG:
                    k0, p0 = pending.pop(0)
                    emit_ctx(ctx_ps, k0, p0)
            if qb < NQB - 1:
                for k0, p0 in pending:
                    emit_ctx(ctx_ps, k0, p0)
                prev_div = (qb, ctx_ps)
            else:
                # final qb: drain head-by-head so divisions start ASAP and
                # the out-projection isn't gated on one long division tail
                for h in range(HPC_):
                    for k0, p0 in pending:
                        nc.tensor.matmul(
                            ctx_ps[h],
                            v2_sb[:, h, k0, 0:HEAD_DIM + 1],
                            p0[h],
                            start=(k0 == 0),
                            stop=(k0 == NKT - 1),
                            skip_group_check=True,
                        )
                    emit_division_head(qb, ctx_ps, h)
                prev_div = None

    if debug_taps:
        nc.sync.dma_start(out=aps["dbg_ctxT2"], in_=ctxT2_sb)

    # --- out-projection (phase 3) ---
    with tc.tile_pool(name="outpsum", bufs=4, space="PSUM") as outp, \
         tc.tile_pool(name="outstg", bufs=4) as outstg:
        for qt in range(S_ // 128):
            for ob in range(NOB):
                op = outp.tile([128, OB_W], F32, tag="op")
                for j in range(PAIRS):
                    nc.tensor.matmul(
                        op,
                        ctxT2_sb[:, j, qt * 128:(qt + 1) * 128],
                        wo_sb[:, j, ob * OB_W:(ob + 1) * OB_W],
                        start=(j == 0),
                        stop=(j == PAIRS - 1),
                    )
                ostg = outstg.tile([128, OB_W], F32, tag="ostg")
                nc.vector.tensor_copy(ostg, op)
                nc.sync.dma_start(
                    out=out_d[qt * 128:(qt + 1) * 128, ob * OB_W:(ob + 1) * OB_W],
                    in_=ostg,
                )

    stack.close()


def build_nc(*, S_=S, C_=HIDDEN, HPC_=HPC, QB=512, with_mask=False,
             enable_asserts=False, debug_taps=False):
    nc = bacc.Bacc(
        "TRN2", target_bir_lowering=False, debug=False,
        enable_asserts=enable_asserts,
    )
    D2 = HPC_ * HEAD_DIM
    PAIRS = HPC_ // 2
    NKT = S_ // 128
    aps = {}
    aps["xbf"] = nc.dram_tensor("xbf", [C_, S_], BF16, kind="ExternalInput").ap()
    aps["xh8"] = nc.dram_tensor("xh8", [C_, S_], FP8, kind="ExternalInput").ap()
    for n in ("wq", "wk", "wv"):
        aps[n] = nc.dram_tensor(n, [C_, D2], BF16, kind="ExternalInput").ap()
    aps["wo"] = nc.dram_tensor("wo", [D2, C_], BF16, kind="ExternalInput").ap()
    for n in ("bq", "bk", "bv"):
        aps[n] = nc.dram_tensor(n, [D2, 1], F32, kind="ExternalInput").ap()
    if with_mask:
        aps["mask01"] = nc.dram_tensor(
            "mask01", [S_, S_], BF16, kind="ExternalInput").ap()
    aps["out"] = nc.dram_tensor("out", [S_, C_], F32, kind="ExternalOutput").ap()
    if debug_taps:
        aps["dbg_qT"] = nc.dram_tensor(
            "dbg_qT", [128, PAIRS, S_], BF16, kind="ExternalOutput").ap()
        aps["dbg_kT"] = nc.dram_tensor(
            "dbg_kT", [128, PAIRS, S_], BF16, kind="ExternalOutput").ap()
        aps["dbg_v2"] = nc.dram_tensor(
            "dbg_v2", [128, HPC_, NKT, HEAD_DIM + 1], BF16,
            kind="ExternalOutput").ap()
        aps["dbg_ctxT2"] = nc.dram_tensor(
            "dbg_ctxT2", [128, PAIRS, S_], BF16, kind="ExternalOutput").ap()
        aps["dbg_en0"] = nc.dram_tensor(
            "dbg_en0", [128, QB], BF16, kind="ExternalOutput").ap()
        aps["dbg_pt0"] = nc.dram_tensor(
            "dbg_pt0", [128, QB], BF16, kind="ExternalOutput").ap()
        aps["dbg_r0"] = nc.dram_tensor(
            "dbg_r0", [1, QB], F32, kind="ExternalOutput").ap()

    with tile.TileContext(nc) as tc:
        emit_kernel(tc, aps, S_=S_, C_=C_, HPC_=HPC_, QB=QB,
                    with_mask=with_mask, debug_taps=debug_taps)
    nc.compile()
    return nc


def host_prepare(x, attn_mask, Wq, bq, Wk, bk, Wv, bv, Wo, bo, *,
                 S_=S, C_=HIDDEN, HPC_=HPC, n_cores=N_CORES):
    """Build the per-core input maps. Returns (in_maps, with_mask)."""
    bf = ml_dtypes.bfloat16
    f8 = ml_dtypes.float8_e4m3fn
    x = np.asarray(x, np.float32)
    B_ = x.shape[0]
    groups = n_cores // B_
    Wq = np.asarray(Wq, np.float32); Wk = np.asarray(Wk, np.float32)
    Wv = np.asarray(Wv, np.float32); Wo = np.asarray(Wo, np.float32)
    bq = np.asarray(bq, np.float32); bk = np.asarray(bk, np.float32)
    bv = np.asarray(bv, np.float32)

    inv_sqrt_d = 1.0 / math.sqrt(HEAD_DIM)
    WqT = np.ascontiguousarray((Wq * inv_sqrt_d).T).astype(bf)  # [C, C]
    WkT = np.ascontiguousarray(Wk.T).astype(bf)
    WvT = np.ascontiguousarray(Wv.T).astype(bf)
    WoT = np.ascontiguousarray(Wo.T).astype(bf)                 # [C(c), C(o)]
    bq = bq * inv_sqrt_d

    mask = np.asarray(attn_mask)
    with_mask = bool(mask.any())
    mask01 = None
    if with_mask:
        # reference: where(mask, -inf) -> multiplicative 0/1 on P
        mask01 = np.where(mask, 0.0, 1.0).astype(bf)
        mask01 = np.ascontiguousarray(mask01.T)  # [k, q]

    in_maps = []
    for core in range(n_cores):
        b, g = divmod(core, groups)
        xb = x[b]                                   # [S, C]
        xT = np.ascontiguousarray(xb.T)             # [C, S]
        norms = np.linalg.norm(xb, axis=1)          # [S]
        scale = (FP8_SCALE / np.maximum(norms, 1e-12)).astype(np.float32)
        xh8 = (xT * scale[None, :]).astype(f8)
        ch = slice(g * HPC_ * HEAD_DIM, (g + 1) * HPC_ * HEAD_DIM)
        m = {
            "xbf": xT.astype(bf),
            "xh8": xh8,
            "wq": np.ascontiguousarray(WqT[:, ch]),
            "wk": np.ascontiguousarray(WkT[:, ch]),
            "wv": np.ascontiguousarray(WvT[:, ch]),
            "wo": np.ascontiguousarray(WoT[ch, :]),
            "bq": np.ascontiguousarray(bq[ch]).reshape(-1, 1),
            "bk": np.ascontiguousarray(bk[ch]).reshape(-1, 1),
            "bv": np.ascontiguousarray(bv[ch]).reshape(-1, 1),
        }
        if with_mask:
            m["mask01"] = mask01
        in_maps.append(m)
    return in_maps, with_mask


_NC_CACHE = {}


def _get_nc(with_mask):
    key = with_mask
    if key not in _NC_CACHE:
        _NC_CACHE[key] = build_nc(with_mask=with_mask)
    return _NC_CACHE[key]


LAST_RESULTS = None


def kernel(**inputs):
    global LAST_RESULTS
    in_maps, with_mask = host_prepare(
        inputs["x"], inputs["attn_mask"],
        inputs["Wq"], inputs["bq"], inputs["Wk"], inputs["bk"],
        inputs["Wv"], inputs["bv"], inputs["Wo"], inputs["bo"],
    )
    nc = _get_nc(with_mask)
    res = run_bass_kernel_spmd(nc, in_maps, core_ids=list(range(N_CORES)))
    LAST_RESULTS = res
    bo = np.asarray(inputs["bo"], np.float32)
    out = np.zeros((B, S, HIDDEN), np.float32)
    groups = N_CORES // B
    for core in range(N_CORES):
        b = core // groups
        out[b] += res.results[core]["out"]
    out += bo[None, None, :]
    return out
